# revision 5
# baseline (speedup 1.0000x reference)
"""DASR (dense_cnn) Trainium2 kernel — nn_DASR_5712306504091.

Data-parallel over batch B=16 -> 8 NeuronCores x 2 samples. Per core,
activations live in SBUF as bf16 [128 partitions = 2 samples x 64 ch,
flat padded 66x66 image + guard zones]. 3x3/1x1 convs run on the
TensorEngine as shifted bf16 matmuls per 16-row chunk accumulating in
fp32 PSUM (host-prepacked block-diagonal [128,128] stationaries). The
per-sample dynamic depthwise conv is split across engines: 2 taps as
diagonal-stationary PE matmuls (host-built from the host-evaluated
kernel-generating MLP), 4 taps on DVE and 3 on Pool as per-partition
scalar multiply/add chains over bf16 tiles (packed-bf16 2x/4x DVE
modes). Epilogues (Prelu/bias/gate/residual) run on Scalar+Vector
engines with interior-only strided writes so padding stays zero.
"""
from contextlib import ExitStack

import numpy as np
import ml_dtypes

import concourse.bacc as bacc
import concourse.bass as bass
import concourse.mybir as mybir
import concourse.tile as tile
from concourse.bass_utils import run_bass_kernel_spmd

F32 = mybir.dt.float32
BF16 = mybir.dt.bfloat16
AF = mybir.ActivationFunctionType
ALU = mybir.AluOpType
NPBF = ml_dtypes.bfloat16

G, NB, C, KK, RED, S = 5, 5, 64, 3, 8, 2
MOCO = 256
B, H, W = 16, 64, 64
NCORE = 8
BL = B // NCORE

HP = WP = H + 2          # 66
IMG = HP * WP            # 4356
GUARD = 68
FLAT = IMG + 2 * GUARD   # 4492
TAPS = [dy * WP + dx for dy in (-1, 0, 1) for dx in (-1, 0, 1)]
PE_TAPS = [0, 1]
DVE_TAPS = [2, 3, 4, 5]
POOL_TAPS = [6, 7, 8]
CH16 = [GUARD + (1 + 16 * i) * WP for i in range(4)]
NDAB = G * NB            # 25
NDA = NDAB * 2           # 50

_PROG_CACHE = {}


def _bd(m):
    out = np.zeros((128, 128), np.float32)
    out[0:64, 0:64] = m
    out[64:128, 64:128] = m
    return out


def _bdvec(v):
    return np.concatenate([v, v]).astype(np.float32)


def _lrelu(x):
    return np.where(x > 0, x, 0.1 * x).astype(np.float32)


def host_prep(x, k_v, head_w, head_b, comp_w, da_kw1, da_kw2, da_cw, da_cb,
              ca_w1, ca_w2, dab_cw, dab_cb, grp_w, grp_b, body_w, body_b,
              up_w, up_b, tail_w, tail_b):
    f = np.float32
    head_st = np.zeros((54, 128), f)
    for ti, (dy, dx) in enumerate([(a, b) for a in (-1, 0, 1) for b in (-1, 0, 1)]):
        for s in range(2):
            for c3 in range(3):
                head_st[ti * 6 + s * 3 + c3, 64 * s:64 * s + 64] = \
                    head_w[:, c3, dy + 1, dx + 1]

    dab_st = np.zeros((128, NDAB, 20, 128), f)
    for g in range(G):
        for n in range(NB):
            gn = g * NB + n
            dab_st[:, gn, 0] = _bd(da_cw[g, n, 0].T)
            dab_st[:, gn, 10] = _bd(da_cw[g, n, 1].T)
            for ti in range(9):
                ky, kx = divmod(ti, 3)
                dab_st[:, gn, 1 + ti] = _bd(dab_cw[g, n, 0][:, :, ky, kx].T)
                dab_st[:, gn, 11 + ti] = _bd(dab_cw[g, n, 1][:, :, ky, kx].T)

    grp_st = np.zeros((128, G, 9, 128), f)
    for g in range(G):
        for ti in range(9):
            ky, kx = divmod(ti, 3)
            grp_st[:, g, ti] = _bd(grp_w[g][:, :, ky, kx].T)
    body_st = np.zeros((128, 9, 128), f)
    for ti in range(9):
        ky, kx = divmod(ti, 3)
        body_st[:, ti] = _bd(body_w[:, :, ky, kx].T)

    up_st = np.zeros((128, 4, 9, 128), f)
    for q in range(4):
        for ti in range(9):
            ky, kx = divmod(ti, 3)
            up_st[:, q, ti] = _bd(up_w[64 * q:64 * q + 64, :, ky, kx].T)

    tail_st4 = np.zeros((4, 9, 128, 24), f)
    for o in range(3):
        for c in range(64):
            for dy in (-1, 0, 1):
                for dx in (-1, 0, 1):
                    wv = tail_w[o, c, dy + 1, dx + 1]
                    for ry in range(2):
                        for rx in range(2):
                            ah, ryp = divmod(ry + dy, 2)
                            aw, rxp = divmod(rx + dx, 2)
                            uc = c * 4 + ryp * 2 + rxp
                            q, i = divmod(uc, 64)
                            ti = (ah + 1) * 3 + (aw + 1)
                            ph = ry * 2 + rx
                            for s in range(2):
                                tail_st4[q, ti, 64 * s + i,
                                         s * 12 + o * 4 + ph] += wv
    tail_st = tail_st4.reshape(36, 128, 24).transpose(1, 0, 2).copy()

    nbias = 112
    bias_all = np.zeros((128, nbias), f)
    bias_all[:, 0] = _bdvec(head_b)
    for g in range(G):
        for n in range(NB):
            for j in range(2):
                idx = (g * NB + n) * 2 + j
                bias_all[:, 1 + idx] = _bdvec(da_cb[g, n, j])
                bias_all[:, 51 + idx] = _bdvec(dab_cb[g, n, j])
    for g in range(G):
        bias_all[:, 101 + g] = _bdvec(grp_b[g])
    bias_all[:, 106] = _bdvec(body_b)
    for q in range(4):
        bias_all[:, 107 + q] = _bdvec(up_b[64 * q:64 * q + 64])
    for s in range(2):
        for o in range(3):
            for ph in range(4):
                bias_all[s * 12 + o * 4 + ph, 111] = tail_b[o]

    shared = dict(head_st=head_st.astype(NPBF),
                  dab_st=dab_st.astype(NPBF),
                  grp_st=grp_st.astype(NPBF), body_st=body_st.astype(NPBF),
                  up_st=up_st.astype(NPBF), tail_st=tail_st.astype(NPBF),
                  bias_all=bias_all)

    # host-evaluated dynamic-kernel / attention MLPs
    kv_full = _lrelu(k_v @ comp_w.T)          # [B, 64]
    eye = np.eye(128, dtype=f)
    per_core = []
    for core in range(NCORE):
        xs = x[core * BL:(core + 1) * BL]
        xpad = np.zeros((2, 3, HP, WP), f)
        xpad[:, :, 1:65, 1:65] = xs
        xflat = xpad.reshape(2, 3, IMG)
        x9 = np.zeros((54, IMG), f)
        for ti, t in enumerate(TAPS):
            lo, hi = max(0, -t), min(IMG, IMG - t)
            for s in range(2):
                for c3 in range(3):
                    x9[ti * 6 + s * 3 + c3, lo:hi] = xflat[s, c3, lo + t:hi + t]
        kv = kv_full[core * BL:(core + 1) * BL]       # [2, 64]
        ker_all = np.zeros((128, NDA * 9), f)
        att_all = np.zeros((128, NDA), f)
        for g in range(G):
            for n in range(NB):
                for j in range(2):
                    idx = (g * NB + n) * 2 + j
                    ker = (_lrelu(kv @ da_kw1[g, n, j].T) @ da_kw2[g, n, j].T)
                    ker = ker.reshape(2, 64, 9)       # [s, c, tap]
                    att = 1.0 / (1.0 + np.exp(-(_lrelu(kv @ ca_w1[g, n, j].T)
                                                @ ca_w2[g, n, j].T)))
                    for s in range(2):
                        ker_all[s * 64:(s + 1) * 64,
                                idx * 9:(idx + 1) * 9] = ker[s]
                        att_all[s * 64:(s + 1) * 64, idx] = att[s]
        kersel = ker_all.reshape(128, NDA, 9)[:, :, PE_TAPS]   # [128, 50, 2]
        dws_pe = (eye[:, None, None, :] * kersel[:, :, :, None]).astype(NPBF)
        per_core.append(dict(x9=x9.astype(NPBF), ker_all=ker_all,
                             att_all=att_all, dws_pe=dws_pe))
    return shared, per_core


def build_program():
    nc = bacc.Bacc("TRN2", target_bir_lowering=False)
    d = {}

    def din(name, shape, dt):
        d[name] = nc.dram_tensor(name, shape, dt, kind="ExternalInput")

    din("x9", [54, IMG], BF16)
    din("head_st", [54, 128], BF16)
    din("dab_st", [128, NDAB, 20, 128], BF16)
    din("dws_pe", [128, NDA, 2, 128], BF16)
    din("grp_st", [128, G, 9, 128], BF16)
    din("body_st", [128, 9, 128], BF16)
    din("up_st", [128, 4, 9, 128], BF16)
    din("tail_st", [128, 36, 24], BF16)
    din("ker_all", [128, NDA * 9], F32)
    din("att_all", [128, NDA], F32)
    din("bias_all", [128, 112], F32)
    y_out = nc.dram_tensor("y_out", [24, 64, 64], F32, kind="ExternalOutput")

    with tile.TileContext(nc) as tc, ExitStack() as ctx:
        acts = ctx.enter_context(tc.tile_pool(name="acts", bufs=1))
        accs = ctx.enter_context(tc.tile_pool(name="accs", bufs=1))
        prodp = ctx.enter_context(tc.tile_pool(name="prod", bufs=1))
        wstp = ctx.enter_context(tc.tile_pool(name="wst", bufs=2))
        once = ctx.enter_context(tc.tile_pool(name="once", bufs=1))
        pp = ctx.enter_context(
            tc.tile_pool(name="ps", bufs=3, space=bass.MemorySpace.PSUM))

        # ---------------- static tiles ----------------
        x0 = acts.tile([128, FLAT], BF16, tag="x0")
        gin = acts.tile([128, FLAT], BF16, tag="gin")
        res = acts.tile([128, FLAT], BF16, tag="res")
        t1 = acts.tile([128, FLAT], BF16, tag="t1")
        t2 = acts.tile([128, FLAT], BF16, tag="t2")
        for i, t in enumerate((x0, res, t1)):
            nc.vector.memset(t[:], 0.0)
        for t in (gin, t2):
            nc.gpsimd.memset(t[:], 0.0)
        accD = accs.tile([128, 4096], BF16, tag="accD")
        accP = accs.tile([128, 4096], BF16, tag="accP")

        x9t = once.tile([54, IMG], BF16, tag="x9")
        nc.sync.dma_start(x9t[:], d["x9"][:])
        head_s = once.tile([54, 128], BF16, tag="headst")
        nc.sync.dma_start(head_s[:], d["head_st"][:])
        bias = once.tile([128, 112], F32, tag="bias")
        nc.sync.dma_start(bias[:], d["bias_all"][:])
        kerT = once.tile([128, NDA * 9], F32, tag="kerall")
        nc.sync.dma_start(kerT[:], d["ker_all"][:])
        attT = once.tile([128, NDA], F32, tag="attall")
        nc.sync.dma_start(attT[:], d["att_all"][:])

        # ---------------- view helpers ----------------
        def mv8(src, q, t):
            v = src[:, q + t:q + t + 8 * WP].rearrange("p (a b) -> p a b", a=8)
            return v[:, :, 1:65]

        def iv16(t_, q0, t=0):
            v = t_[:, q0 + t:q0 + t + 16 * WP].rearrange("p (a b) -> p a b", a=16)
            return v[:, :, 1:65]

        def cv16(comp, ci):
            return comp[:, ci * 1024:(ci + 1) * 1024].rearrange(
                "p (a b) -> p a b", a=16)

        def cvf(comp):
            return comp[:].rearrange("p (a b) -> p a b", a=64)

        def ivf(t_, t=0):
            v = t_[:, GUARD + WP + t:GUARD + WP + t + 64 * WP].rearrange(
                "p (a b) -> p a b", a=64)
            return v[:, :, 1:65]

        def ps16(ps):
            return ps[:].rearrange("p (a b) -> p a b", a=16)

        # ---------------- conv helpers ----------------
        def conv3x3(src, dst, st3, bias_col, act=True, res_add=None):
            inplace = src is dst
            pend = None

            def epilogue(ps, q0):
                if act:
                    nc.scalar.activation(iv16(dst, q0), ps16(ps),
                                         AF.Prelu, bias=bias_col, alpha=0.1)
                else:
                    nc.vector.scalar_tensor_tensor(
                        iv16(dst, q0), ps16(ps), bias_col,
                        iv16(res_add, q0), op0=ALU.add, op1=ALU.add)

            for q0 in CH16:
                ps = pp.tile([128, 1024], F32, tag="ps")
                for h in range(2):
                    q = q0 + h * 8 * WP
                    out_h = ps[:, h * 512:(h + 1) * 512]
                    for ti, t in enumerate(TAPS):
                        nc.tensor.matmul(out_h, st3[:, ti, :], mv8(src, q, t),
                                         start=(ti == 0), stop=(ti == 8))
                if not inplace:
                    epilogue(ps, q0)
                else:
                    if pend is not None:
                        epilogue(*pend)
                    pend = (ps, q0)
            if pend is not None:
                epilogue(*pend)

        def conv1x1_gate(src, xres, dst, st1, idx):
            att_col = attT[:, idx:idx + 1]
            cb_col = bias[:, 1 + idx:2 + idx]
            for ci, q0 in enumerate(CH16):
                ps = pp.tile([128, 1024], F32, tag="ps")
                for h in range(2):
                    q = q0 + h * 8 * WP
                    nc.tensor.matmul(ps[:, h * 512:(h + 1) * 512], st1,
                                     mv8(src, q, 0), start=True, stop=True)
                vi = cv16(accD, ci)
                nc.vector.scalar_tensor_tensor(
                    vi, iv16(xres, q0), att_col, ps16(ps),
                    op0=ALU.mult, op1=ALU.add)
                nc.scalar.activation(iv16(dst, q0), vi, AF.Prelu,
                                     bias=cb_col, alpha=0.1)

        def dw_conv(src, dst, idx, dws):
            def kc(tap):
                j = idx * 9 + tap
                return kerT[:, j:j + 1]

            tms = [prodp.tile([128, 4096], BF16, tag=f"tmp{k}",
                              name=f"tmp{k}")
                   for k in range(len(DVE_TAPS))]
            ptms = [prodp.tile([128, 4096], BF16, tag=f"ptmp{k}",
                               name=f"ptmp{k}")
                    for k in range(len(POOL_TAPS) - 1)]
            for ci, q0 in enumerate(CH16):
                ps = pp.tile([128, 1024], F32, tag="ps")
                for h in range(2):
                    q = q0 + h * 8 * WP
                    out_h = ps[:, h * 512:(h + 1) * 512]
                    for j, tap in enumerate(PE_TAPS):
                        nc.tensor.matmul(out_h, dws[:, j, :],
                                         mv8(src, q, TAPS[tap]),
                                         start=(j == 0), stop=(j == 1))
                # products (independent of acc) on DVE + Pool
                for k, tap in enumerate(DVE_TAPS):
                    nc.vector.tensor_scalar_mul(
                        cv16(tms[k], ci), iv16(src, q0, TAPS[tap]), kc(tap))
                nc.gpsimd.tensor_scalar_mul(
                    cv16(accP, ci), iv16(src, q0, TAPS[POOL_TAPS[0]]),
                    kc(POOL_TAPS[0]))
                for k, tap in enumerate(POOL_TAPS[1:]):
                    nc.gpsimd.tensor_scalar_mul(
                        cv16(ptms[k], ci), iv16(src, q0, TAPS[tap]), kc(tap))
                # init acc from PSUM, then accumulate
                nc.scalar.activation(cv16(accD, ci), ps16(ps), AF.Identity)
                for tm in tms:
                    nc.vector.tensor_tensor(cv16(accD, ci), cv16(accD, ci),
                                            cv16(tm, ci), op=ALU.add)
                for tm in ptms:
                    nc.gpsimd.tensor_tensor(cv16(accP, ci), cv16(accP, ci),
                                            cv16(tm, ci), op=ALU.add)
                nc.gpsimd.tensor_tensor(cv16(accD, ci), cv16(accD, ci),
                                        cv16(accP, ci), op=ALU.add)
                nc.scalar.activation(iv16(dst, q0), cv16(accD, ci),
                                     AF.Prelu, alpha=0.1)

        # ---------------- head conv ----------------
        for ci, q0 in enumerate(CH16):
            ps = pp.tile([128, 1024], F32, tag="ps")
            for h in range(2):
                q = q0 - GUARD + h * 8 * WP
                v = x9t[:, q:q + 8 * WP].rearrange("p (a b) -> p a b", a=8)
                nc.tensor.matmul(ps[:, h * 512:(h + 1) * 512], head_s[:],
                                 v[:, :, 1:65], start=True, stop=True)
            nc.scalar.activation(iv16(x0, q0), ps16(ps), AF.Identity,
                                 bias=bias[:, 0:1])
        nc.vector.tensor_copy(ivf(res), ivf(x0))
        nc.vector.tensor_copy(ivf(gin), ivf(x0))

        # ---------------- body ----------------
        for g in range(G):
            for n_ in range(NB):
                gn = g * NB + n_
                wst = wstp.tile([128, 20, 128], BF16, tag="wst")
                nc.sync.dma_start(wst[:], d["dab_st"][:, gn])
                ia, ib = gn * 2, gn * 2 + 1
                dwsA = wstp.tile([128, 2, 128], BF16, tag="dws")
                nc.sync.dma_start(dwsA[:], d["dws_pe"][:, ia])
                dwsB = wstp.tile([128, 2, 128], BF16, tag="dws")
                nc.sync.dma_start(dwsB[:], d["dws_pe"][:, ib])
                dw_conv(res, t1, ia, dwsA)
                conv1x1_gate(t1, res, t1, wst[:, 0, :], ia)
                conv3x3(t1, t2, wst[:, 1:10, :], bias[:, 51 + ia:52 + ia])
                dw_conv(t2, t1, ib, dwsB)
                conv1x1_gate(t1, t2, t1, wst[:, 10, :], ib)
                conv3x3(t1, res, wst[:, 11:20, :], bias[:, 51 + ib:52 + ib],
                        act=False, res_add=res)
            gst = wstp.tile([128, 9, 128], BF16, tag="gst")
            nc.sync.dma_start(gst[:], d["grp_st"][:, g])
            conv3x3(res, res, gst[:, :, :], bias[:, 101 + g:102 + g],
                    act=False, res_add=gin)
            if g < G - 1:
                nc.vector.tensor_copy(ivf(gin), ivf(res))

        bst = wstp.tile([128, 9, 128], BF16, tag="gst")
        nc.sync.dma_start(bst[:], d["body_st"][:])
        conv3x3(res, res, bst[:, :, :], bias[:, 106:107],
                act=False, res_add=x0)

        # ---------------- upsampler ----------------
        ust = once.tile([128, 4, 9, 128], BF16, tag="ust")
        nc.sync.dma_start(ust[:], d["up_st"][:])
        uts = []
        for q_, tg in enumerate(("gin", "t1", "t2", "x0")):
            ut = acts.tile([128, FLAT], BF16, tag=tg)
            if q_ % 2 == 0:
                nc.vector.memset(ut[:], 0.0)
            else:
                nc.gpsimd.memset(ut[:], 0.0)
            uts.append(ut)
            for q0 in CH16:
                ps = pp.tile([128, 1024], F32, tag="ps")
                for h in range(2):
                    q = q0 + h * 8 * WP
                    out_h = ps[:, h * 512:(h + 1) * 512]
                    for ti, t in enumerate(TAPS):
                        nc.tensor.matmul(out_h, ust[:, q_, ti, :],
                                         mv8(res, q, t),
                                         start=(ti == 0), stop=(ti == 8))
                nc.scalar.activation(iv16(ut, q0), ps16(ps), AF.Identity,
                                     bias=bias[:, 107 + q_:108 + q_])

        # ---------------- tail + pixel shuffle ----------------
        tst = once.tile([128, 36, 24], BF16, tag="tst")
        nc.sync.dma_start(tst[:], d["tail_st"][:])
        otail = acts.tile([24, 4096], F32, tag="otail")
        for ci, q0 in enumerate(CH16):
            ps = pp.tile([24, 1024], F32, tag="ps")
            for h in range(2):
                q = q0 + h * 8 * WP
                out_h = ps[:, h * 512:(h + 1) * 512]
                k = 0
                for q_ in range(4):
                    for ti, t in enumerate(TAPS):
                        nc.tensor.matmul(out_h, tst[:, q_ * 9 + ti, :],
                                         mv8(uts[q_], q, t),
                                         start=(k == 0), stop=(k == 35))
                        k += 1
            ov = otail[:, ci * 1024:(ci + 1) * 1024].rearrange(
                "p (a b) -> p a b", a=16)
            nc.scalar.activation(ov, ps16(ps), AF.Identity,
                                 bias=bias[0:24, 111:112])

        # ---------------- output DMA (phase-major; host de-shuffles) -------
        nc.sync.dma_start(y_out[:],
                          otail[:].rearrange("p (a b) -> p a b", a=64))

    nc.compile()
    return nc


def kernel(**inputs):
    inputs = {k: np.asarray(v, dtype=np.float32) for k, v in inputs.items()}
    shared, per_core = host_prep(**inputs)

    if "nc" not in _PROG_CACHE:
        _PROG_CACHE["nc"] = build_program()
    nc = _PROG_CACHE["nc"]

    in_maps = [{**shared, **pc} for pc in per_core]
    last_err = None
    for _attempt in range(3):
        try:
            res = run_bass_kernel_spmd(nc, in_maps, core_ids=list(range(NCORE)))
            break
        except Exception as e:
            last_err = e
    else:
        raise last_err

    out = np.zeros((B, 3, 2 * H, 2 * W), np.float32)
    for core in range(NCORE):
        ph = res.results[core]["y_out"].reshape(2, 3, 2, 2, 64, 64)
        out[core * BL:(core + 1) * BL] = \
            ph.transpose(0, 1, 4, 2, 5, 3).reshape(2, 3, 128, 128)
    return out
